# revision 18
# baseline (speedup 1.0000x reference)
"""Trainium2 Bass kernel for nn_GCNNDoubleQCritic (gnn_message_passing).

Key insight: the reference's knn is degenerate — `jnp.eye(N)*inf` makes the
whole distance matrix NaN (0*inf=NaN), and jax top_k sorts NaNs first (ties by
lower index). So node i's "neighbors" are simply the first 5 indices != i, the
in-degree is exactly 5 everywhere (deg=6 with self loop), and the GCN layer
collapses to, per sample:

    z   = h @ W
    c5  = z[0]+z[1]+z[2]+z[3]+z[4]
    out_i = relu((z_i + c5)/6 + b)   for i >= 6
    out_i = out_5                    for i <= 5     (since (z_5+c5) = sum_{j<=5} z_j)

The 1/6 is folded into the next layer's weights (relu is positively
homogeneous), so the on-chip layer is: G = relu(z + c5 + 6b), with W_{l>0} and
mlp_w pre-scaled by 1/6 on-chip.

Sharding: data-parallel over batch — 8 cores x 16 samples. Activations are
kept feature-major ([H, nodes]) the whole way so every matmul is a natural
lhsT=W[fi,fo], rhs=h_T[fi, n] contraction with zero transposes; the only
layout change (x -> x_T) is done host-side as part of input marshalling.
"""

import sys

sys.path.insert(0, "/opt/trn_rl_repo")

import numpy as np

B, N = 128, 1024
NCORES = 8
S = B // NCORES  # samples per core
NNODES = S * N  # nodes per core
H = 128
FIN = 10

# 'fp32' (exact, 4 cyc/row matmuls), 'f32r' (exact, 1 cyc/row), 'bf16'
MODE = "f32r"

_CACHE = {}
LAST_RESULTS = None


def _build(mode):
    import concourse.bacc as bacc
    import concourse.mybir as mybir
    from concourse.tile import TileContext

    fp = mybir.dt.float32
    bf = mybir.dt.bfloat16
    if mode == "bf16":
        act_dt = bf  # dtype h/x/weights are stored in (matmul operand dtype)
    elif mode == "f32r":
        act_dt = mybir.dt.float32r
    else:
        act_dt = fp

    nc = bacc.Bacc(
        "TRN2",
        target_bir_lowering=False,
        debug=False,
        enable_asserts=False,
        num_devices=NCORES,
    )

    in_dt = bf if mode == "bf16" else fp
    xT_d = nc.dram_tensor("xT", [128, NNODES // 4], in_dt, kind="ExternalInput")
    w_d = {}
    b_d = {}
    for st in ("q1", "q2"):
        for li in range(3):
            fi = FIN if li == 0 else H
            w_d[st, li] = nc.dram_tensor(f"{st}_w{li}", [fi, H], fp, kind="ExternalInput")
            b_d[st, li] = nc.dram_tensor(f"{st}_b{li}", [H, 1], fp, kind="ExternalInput")
    mlpw_d = nc.dram_tensor("mlp_w", [H, 1], fp, kind="ExternalInput")
    mlpb_d = nc.dram_tensor("mlp_b", [1, 1], fp, kind="ExternalInput")
    q_d = {
        st: nc.dram_tensor(f"{st}_out", [4, 4096], fp, kind="ExternalOutput")
        for st in ("q1", "q2")
    }

    with TileContext(nc) as tc:
        with (
            tc.tile_pool(name="const", bufs=1) as const,
            tc.tile_pool(name="hbuf", bufs=1) as hbuf,
            tc.tile_pool(name="small", bufs=8) as small,
            tc.tile_pool(name="zpsum", bufs=3, space="PSUM") as zpsum,
            tc.tile_pool(name="mpsum", bufs=2, space="PSUM") as mpsum,
        ):
            # ---- load constants ----
            # xT packed 4 quarters deep: xTp[32*q + f, m] = x_T[f, q*NQ + m]
            NQ = NNODES // 4
            if mode == "f32r":
                xTraw = const.tile([128, NQ], fp, tag="xTraw")
                nc.sync.dma_start(xTraw[:], xT_d.ap())
                xTp = const.tile([128, NQ], act_dt, tag="xTp")
                nc.vector.tensor_copy(xTp[:], xTraw[:])
            else:
                xTp = const.tile([128, NQ], act_dt, tag="xTp")
                nc.sync.dma_start(xTp[:], xT_d.ap())

            w_sb = {}
            b6_sb = {}
            for st in ("q1", "q2"):
                for li in range(3):
                    fi = FIN if li == 0 else H
                    wraw = const.tile([fi, H], fp, tag=f"wraw_{st}{li}")
                    nc.sync.dma_start(wraw[:], w_d[st, li].ap())
                    if li == 0:
                        # replicate w0 at partition offsets 0/32/64/96 for row tiling
                        wk = const.tile([128, H], act_dt, tag=f"w_{st}{li}")
                        for q in range(4):
                            nc.vector.tensor_copy(wk[32 * q : 32 * q + fi, :], wraw[:])
                    else:
                        wk = const.tile([fi, H], act_dt, tag=f"w_{st}{li}")
                        nc.vector.tensor_scalar_mul(wk[:], wraw[:], 1.0 / 6.0)
                    w_sb[st, li] = wk
                    braw = const.tile([H, 1], fp, tag=f"braw_{st}{li}")
                    nc.sync.dma_start(braw[:], b_d[st, li].ap())
                    b6 = const.tile([H, 1], fp, tag=f"b6_{st}{li}")
                    nc.vector.tensor_scalar_mul(b6[:], braw[:], 6.0)
                    b6_sb[st, li] = b6
            mwraw = const.tile([H, 1], fp, tag="mwraw")
            nc.sync.dma_start(mwraw[:], mlpw_d.ap())
            mwscl = const.tile([H, 1], fp, tag="mwscl")
            nc.vector.tensor_scalar_mul(mwscl[:], mwraw[:], 1.0 / 6.0)
            # mlp weight replicated across all 128 stationary columns: every
            # psum row of the head matmul equals q for that chunk
            mwrep = const.tile([H, H], act_dt, tag="mwrep")
            nc.vector.tensor_copy(mwrep[:], mwscl[:].to_broadcast([H, H]))
            mb128 = const.tile([128, 1], fp, tag="mb128")
            nc.sync.dma_start(mb128[:], mlpb_d.ap().to_broadcast([128, 1]))

            hA = hbuf.tile([H, NNODES], act_dt, tag="hA")
            hB = hbuf.tile([H, NNODES], act_dt, tag="hB")
            qsb = const.tile([128, 4096], fp, tag="qsb", name="qsb")

            for st in ("q1", "q2"):
                for li in range(3):
                    h_in = (hA if li % 2 == 1 else hB) if li > 0 else None
                    h_out = hA if li % 2 == 0 else hB
                    wk = w_sb[st, li]
                    b6 = b6_sb[st, li]
                    for s in range(S):
                        zT = zpsum.tile([H, N], fp, tag="zT")
                        for half in range(2):
                            if li == 0:
                                q4 = s // 4
                                c0 = (s % 4) * N + half * 512
                                nc.tensor.matmul(
                                    zT[:, half * 512 : (half + 1) * 512],
                                    wk[32 * q4 : 32 * q4 + FIN, :],
                                    xTp[32 * q4 : 32 * q4 + FIN, c0 : c0 + 512],
                                    start=True,
                                    stop=True,
                                    tile_position=(32 * q4, 0),
                                )
                            else:
                                c0 = s * N + half * 512
                                nc.tensor.matmul(
                                    zT[:, half * 512 : (half + 1) * 512],
                                    wk[:],
                                    h_in[:, c0 : c0 + 512],
                                    start=True,
                                    stop=True,
                                )
                        bias_s = small.tile([H, 1], fp, tag="bias_s")
                        # bias_s = sum(z cols 0..4) + 6b
                        c5 = small.tile([H, 1], fp, tag="c5")
                        nc.vector.tensor_reduce(
                            c5[:], zT[:, 0:5], axis=mybir.AxisListType.X, op=mybir.AluOpType.add
                        )
                        nc.vector.tensor_add(bias_s[:], c5[:], b6[:])
                        seg = h_out[:, s * N : (s + 1) * N]
                        if s % 2 == 0:
                            nc.scalar.activation(
                                seg,
                                zT[:],
                                mybir.ActivationFunctionType.Relu,
                                bias=bias_s[:],
                                scale=1.0,
                            )
                        else:
                            nc.vector.tensor_scalar(
                                seg,
                                zT[:],
                                bias_s[:],
                                0.0,
                                op0=mybir.AluOpType.add,
                                op1=mybir.AluOpType.max,
                            )
                        # nodes 0..4 take node 5's value
                        nc.vector.tensor_copy(
                            h_out[:, s * N : s * N + 5],
                            h_out[:, s * N + 5 : s * N + 6].to_broadcast([H, 5]),
                        )
                    h_in = h_out

                # ---- output head: q = h3 @ (mlp_w/6) + mlp_b ----
                h3 = hA
                for c in range(NNODES // 512):  # 32 chunks
                    mp = mpsum.tile([H, 512], fp, tag="mp")
                    nc.tensor.matmul(
                        mp[:], mwrep[:], h3[:, c * 512 : (c + 1) * 512], start=True, stop=True
                    )
                    # every row of mp equals q[chunk c]; evac from a 32-aligned
                    # partition (PSUM engine access must be 32-aligned)
                    p = 32 * (c % 4)
                    g = c // 4
                    if c % 2 == 0:
                        nc.vector.tensor_scalar_add(
                            qsb[p : p + 1, g * 512 : (g + 1) * 512],
                            mp[p : p + 1, :],
                            mb128[p : p + 1, :],
                        )
                    else:
                        nc.scalar.activation(
                            qsb[p : p + 1, g * 512 : (g + 1) * 512],
                            mp[p : p + 1, :],
                            mybir.ActivationFunctionType.Identity,
                            bias=mb128[p : p + 1, :],
                            scale=1.0,
                        )
                for j in range(4):
                    nc.sync.dma_start(q_d[st].ap()[j : j + 1, :], qsb[32 * j : 32 * j + 1, :])

    nc.compile()
    return nc


def _make_runner(nc):
    """Like bass2jax.run_bass_via_pjrt, but returns a reusable jitted callable
    so repeat calls skip retracing/relowering."""
    import jax
    import numpy as np
    from jax.sharding import Mesh, PartitionSpec
    from jax.experimental.shard_map import shard_map

    import concourse.mybir as mybir
    from concourse import bass2jax

    bass2jax.install_neuronx_cc_hook()

    partition_name = nc.partition_id_tensor.name if nc.partition_id_tensor else None
    in_names, out_names, out_avals, zero_outs = [], [], [], []
    for alloc in nc.m.functions[0].allocations:
        if not isinstance(alloc, mybir.MemoryLocationSet):
            continue
        name = alloc.memorylocations[0].name
        if alloc.kind == "ExternalInput":
            if name != partition_name:
                in_names.append(name)
        elif alloc.kind == "ExternalOutput":
            out_names.append(name)
            shape = tuple(alloc.tensor_shape)
            dtype = mybir.dt.np(alloc.dtype)
            out_avals.append(jax.core.ShapedArray(shape, dtype))
            zero_outs.append(np.zeros(shape, dtype))
    n_params = len(in_names)
    n_outs = len(out_avals)
    all_in_names = list(in_names) + list(out_names)
    if partition_name is not None:
        all_in_names.append(partition_name)

    def _body(*args):
        operands = list(args)
        if partition_name is not None:
            operands.append(bass2jax.partition_id_tensor())
        outs = bass2jax._bass_exec_p.bind(
            *operands,
            out_avals=tuple(out_avals),
            in_names=tuple(all_in_names),
            out_names=tuple(out_names),
            lowering_input_output_aliases=(),
            sim_require_finite=True,
            sim_require_nnan=True,
            nc=nc,
        )
        return tuple(outs)

    devices = jax.devices()[:NCORES]
    mesh = Mesh(np.asarray(devices), ("core",))
    sharded = jax.jit(
        shard_map(
            _body,
            mesh=mesh,
            in_specs=(PartitionSpec("core"),) * (n_params + n_outs),
            out_specs=(PartitionSpec("core"),) * n_outs,
            check_rep=False,
        ),
        keep_unused=True,
    )

    def run(in_maps):
        concat_in = [
            np.concatenate([np.asarray(m[name]) for m in in_maps], axis=0)
            for name in in_names
        ]
        concat_zeros = [
            np.zeros((NCORES * z.shape[0], *z.shape[1:]), z.dtype) for z in zero_outs
        ]
        out_arrs = sharded(*concat_in, *concat_zeros)
        return [
            {
                name: np.asarray(out_arrs[i]).reshape(NCORES, *out_avals[i].shape)[c]
                for i, name in enumerate(out_names)
            }
            for c in range(NCORES)
        ]

    run.sharded = sharded
    run.in_names = in_names
    run.out_names = out_names
    run.out_avals = out_avals
    run.zero_outs = zero_outs
    return run


def kernel(**inputs):
    global LAST_RESULTS
    import os

    obs = np.asarray(inputs["obs"], dtype=np.float32).reshape(B, N, FIN)
    act = np.asarray(inputs["action"], dtype=np.float32).reshape(B, N, 2)
    x = np.concatenate([obs[..., 2:], act], axis=-1)  # [B, N, 10]
    xT = x.transpose(2, 0, 1).reshape(FIN, B * N)  # [10, B*N]

    if MODE == "bf16":
        import ml_dtypes

        xT = xT.astype(ml_dtypes.bfloat16)

    # per-core packed layout: xTp[32*q + f, m] = xT_core[f, q*NQ + m]
    NQ = NNODES // 4
    xTp_cores = []
    for c in range(NCORES):
        xc = xT[:, c * NNODES : (c + 1) * NNODES].reshape(FIN, 4, NQ)
        packed = np.zeros((128, NQ), dtype=xT.dtype)
        for q in range(4):
            packed[32 * q : 32 * q + FIN, :] = xc[:, q, :]
        xTp_cores.append(packed)

    if MODE not in _CACHE:
        nc = _build(MODE)
        _CACHE[MODE] = (nc, _make_runner(nc))
    nc, runner = _CACHE[MODE]

    base = {
        "mlp_w": np.asarray(inputs["mlp_w"], np.float32).reshape(H, 1),
        "mlp_b": np.asarray(inputs["mlp_b"], np.float32).reshape(1, 1),
    }
    for st in ("q1", "q2"):
        for li in range(3):
            base[f"{st}_w{li}"] = np.ascontiguousarray(np.asarray(inputs[f"{st}_w{li}"], np.float32))
            base[f"{st}_b{li}"] = np.asarray(inputs[f"{st}_b{li}"], np.float32).reshape(H, 1)

    in_maps = []
    for c in range(NCORES):
        m = dict(base)
        m["xT"] = xTp_cores[c]
        in_maps.append(m)

    results = runner(in_maps)
    LAST_RESULTS = results

    def unpack(r, name):
        # row j holds chunks c with c%4==j at free offset (c//4)*512
        o = np.asarray(r[name]).reshape(4, 8, 512)
        return o.transpose(1, 0, 2).reshape(S, N)

    q1 = np.concatenate([unpack(r, "q1_out") for r in results], axis=0)
    q2 = np.concatenate([unpack(r, "q2_out") for r in results], axis=0)
    return q1.astype(np.float32), q2.astype(np.float32)


if __name__ == "__main__":
    import reference as ref

    inputs = {k: np.asarray(v) for k, v in ref.setup_inputs().items()}
    q1, q2 = kernel(**inputs)
    print(q1[0, :8])


# revision 21
# speedup vs baseline: 4.1300x; 4.1300x over previous
"""Trainium2 Bass kernel for nn_GCNNDoubleQCritic (gnn_message_passing).

Key insight: the reference's knn is degenerate — `jnp.eye(N)*inf` makes the
whole distance matrix NaN (0*inf=NaN), and jax top_k sorts NaNs first (ties by
lower index). So node i's "neighbors" are simply the first 5 indices != i, the
in-degree is exactly 5 everywhere (deg=6 with self loop), and the GCN layer
collapses to, per sample:

    z   = h @ W
    c5  = z[0]+z[1]+z[2]+z[3]+z[4]
    out_i = relu((z_i + c5)/6 + b)   for i >= 6
    out_i = out_5                    for i <= 5     (since (z_5+c5) = sum_{j<=5} z_j)

The 1/6 is folded into the next layer's weights (relu is positively
homogeneous), so the on-chip layer is: G = relu(z + c5 + 6b), with W_{l>0} and
mlp_w pre-scaled by 1/6 on-chip.

Sharding: data-parallel over batch — 8 cores x 16 samples. Activations are
kept feature-major ([H, nodes]) the whole way so every matmul is a natural
lhsT=W[fi,fo], rhs=h_T[fi, n] contraction with zero transposes; the only
layout change (x -> x_T) is done host-side as part of input marshalling.
"""

import sys

sys.path.insert(0, "/opt/trn_rl_repo")

import numpy as np

B, N = 128, 1024
NCORES = 8
PACK_COLS = 4096 + 6 * 128 + 8  # xTp | 6 weights | 6 bias cols | mlp_w | mlp_b
S = B // NCORES  # samples per core
NNODES = S * N  # nodes per core
H = 128
FIN = 10

# 'fp32' (exact, 4 cyc/row matmuls), 'f32r' (exact, 1 cyc/row), 'bf16'
MODE = "f32r"

_CACHE = {}
LAST_RESULTS = None


def _build(mode):
    import concourse.bacc as bacc
    import concourse.mybir as mybir
    from concourse.tile import TileContext

    fp = mybir.dt.float32
    bf = mybir.dt.bfloat16
    if mode == "bf16":
        act_dt = bf  # dtype h/x/weights are stored in (matmul operand dtype)
    elif mode == "f32r":
        act_dt = mybir.dt.float32r
    else:
        act_dt = fp

    nc = bacc.Bacc(
        "TRN2",
        target_bir_lowering=False,
        debug=False,
        enable_asserts=False,
        num_devices=NCORES,
    )

    # single packed input: cols [0,4096) = xTp, then per-stack weights
    # (6x128 cols), then 6 bias cols, mlp_w col, mlp_b col (replicated rows)
    in_dt = bf if mode == "bf16" else fp
    inp_d = nc.dram_tensor("inp", [128, PACK_COLS], in_dt, kind="ExternalInput")
    q_d = nc.dram_tensor("q_out", [8, 4096], fp, kind="ExternalOutput")

    with TileContext(nc) as tc:
        with (
            tc.tile_pool(name="const", bufs=1) as const,
            tc.tile_pool(name="hbuf", bufs=1) as hbuf,
            tc.tile_pool(name="small", bufs=8) as small,
            tc.tile_pool(name="zpsum", bufs=3, space="PSUM") as zpsum,
            tc.tile_pool(name="mpsum", bufs=2, space="PSUM") as mpsum,
        ):
            # ---- load the single packed input ----
            NQ = NNODES // 4
            big = const.tile([128, PACK_COLS], in_dt, tag="big")
            nc.sync.dma_start(big[:], inp_d.ap())

            if mode == "f32r":
                xTp = const.tile([128, NQ], act_dt, tag="xTp")
                nc.vector.tensor_copy(xTp[:], big[:, 0:NQ])
            else:
                xTp = big[:, 0:NQ]

            w_sb = {}
            b6_sb = {}
            for idx, st in enumerate(("q1", "q2")):
                for li in range(3):
                    fi = FIN if li == 0 else H
                    wcol = NQ + (idx * 3 + li) * H
                    wsrc = big[0:fi, wcol : wcol + H]
                    if li == 0:
                        # replicate w0 at partition offsets 0/32/64/96 for row tiling
                        wk = const.tile([128, H], act_dt, tag=f"w_{st}{li}")
                        for q in range(4):
                            nc.vector.tensor_copy(wk[32 * q : 32 * q + fi, :], wsrc)
                    else:
                        wk = const.tile([fi, H], act_dt, tag=f"w_{st}{li}")
                        nc.vector.tensor_scalar_mul(wk[:], wsrc, 1.0 / 6.0)
                    w_sb[st, li] = wk
                    bcol = NQ + 6 * H + (idx * 3 + li)
                    b6 = const.tile([H, 1], fp, tag=f"b6_{st}{li}")
                    nc.vector.tensor_scalar_mul(b6[:], big[:, bcol : bcol + 1], 6.0)
                    b6_sb[st, li] = b6
            mwcol = NQ + 6 * H + 6
            mwscl = const.tile([H, 1], fp, tag="mwscl")
            nc.vector.tensor_scalar_mul(mwscl[:], big[:, mwcol : mwcol + 1], 1.0 / 6.0)
            # mlp weight replicated across all 128 stationary columns: every
            # psum row of the head matmul equals q for that chunk
            mwrep = const.tile([H, H], act_dt, tag="mwrep")
            nc.vector.tensor_copy(mwrep[:], mwscl[:].to_broadcast([H, H]))
            # mlp_b replicated into every row host-side
            mb128 = big[:, mwcol + 1 : mwcol + 2]

            hA = hbuf.tile([H, NNODES], act_dt, tag="hA")
            hB = hbuf.tile([H, NNODES], act_dt, tag="hB")
            qsb = const.tile([128, 4096], fp, tag="qsb", name="qsb")

            for st in ("q1", "q2"):
                for li in range(3):
                    h_in = (hA if li % 2 == 1 else hB) if li > 0 else None
                    h_out = hA if li % 2 == 0 else hB
                    wk = w_sb[st, li]
                    b6 = b6_sb[st, li]
                    for s in range(S):
                        zT = zpsum.tile([H, N], fp, tag="zT")
                        for half in range(2):
                            if li == 0:
                                q4 = s // 4
                                c0 = (s % 4) * N + half * 512
                                nc.tensor.matmul(
                                    zT[:, half * 512 : (half + 1) * 512],
                                    wk[32 * q4 : 32 * q4 + FIN, :],
                                    xTp[32 * q4 : 32 * q4 + FIN, c0 : c0 + 512],
                                    start=True,
                                    stop=True,
                                    tile_position=(32 * q4, 0),
                                )
                            else:
                                c0 = s * N + half * 512
                                nc.tensor.matmul(
                                    zT[:, half * 512 : (half + 1) * 512],
                                    wk[:],
                                    h_in[:, c0 : c0 + 512],
                                    start=True,
                                    stop=True,
                                )
                        bias_s = small.tile([H, 1], fp, tag="bias_s")
                        # bias_s = sum(z cols 0..4) + 6b
                        c5 = small.tile([H, 1], fp, tag="c5")
                        nc.vector.tensor_reduce(
                            c5[:], zT[:, 0:5], axis=mybir.AxisListType.X, op=mybir.AluOpType.add
                        )
                        nc.vector.tensor_add(bias_s[:], c5[:], b6[:])
                        seg = h_out[:, s * N : (s + 1) * N]
                        if s % 2 == 0:
                            nc.scalar.activation(
                                seg,
                                zT[:],
                                mybir.ActivationFunctionType.Relu,
                                bias=bias_s[:],
                                scale=1.0,
                            )
                        else:
                            nc.vector.tensor_scalar(
                                seg,
                                zT[:],
                                bias_s[:],
                                0.0,
                                op0=mybir.AluOpType.add,
                                op1=mybir.AluOpType.max,
                            )
                        # nodes 0..4 take node 5's value
                        nc.vector.tensor_copy(
                            h_out[:, s * N : s * N + 5],
                            h_out[:, s * N + 5 : s * N + 6].to_broadcast([H, 5]),
                        )
                    h_in = h_out

                # ---- output head: q = h3 @ (mlp_w/6) + mlp_b ----
                h3 = hA
                for c in range(NNODES // 512):  # 32 chunks
                    mp = mpsum.tile([H, 512], fp, tag="mp")
                    nc.tensor.matmul(
                        mp[:], mwrep[:], h3[:, c * 512 : (c + 1) * 512], start=True, stop=True
                    )
                    # every row of mp equals q[chunk c]; evac from a 32-aligned
                    # partition (PSUM engine access must be 32-aligned)
                    p = 32 * (c % 4)
                    g = c // 4
                    if c % 2 == 0:
                        nc.vector.tensor_scalar_add(
                            qsb[p : p + 1, g * 512 : (g + 1) * 512],
                            mp[p : p + 1, :],
                            mb128[p : p + 1, :],
                        )
                    else:
                        nc.scalar.activation(
                            qsb[p : p + 1, g * 512 : (g + 1) * 512],
                            mp[p : p + 1, :],
                            mybir.ActivationFunctionType.Identity,
                            bias=mb128[p : p + 1, :],
                            scale=1.0,
                        )
                row0 = 0 if st == "q1" else 4
                for j in range(4):
                    nc.sync.dma_start(
                        q_d.ap()[row0 + j : row0 + j + 1, :], qsb[32 * j : 32 * j + 1, :]
                    )

    nc.compile()
    return nc


def _make_runner(nc):
    """Like bass2jax.run_bass_via_pjrt, but returns a reusable jitted callable
    so repeat calls skip retracing/relowering."""
    import jax
    import numpy as np
    from jax.sharding import Mesh, PartitionSpec
    from jax.experimental.shard_map import shard_map

    import concourse.mybir as mybir
    from concourse import bass2jax

    bass2jax.install_neuronx_cc_hook()

    partition_name = nc.partition_id_tensor.name if nc.partition_id_tensor else None
    in_names, out_names, out_avals, zero_outs = [], [], [], []
    for alloc in nc.m.functions[0].allocations:
        if not isinstance(alloc, mybir.MemoryLocationSet):
            continue
        name = alloc.memorylocations[0].name
        if alloc.kind == "ExternalInput":
            if name != partition_name:
                in_names.append(name)
        elif alloc.kind == "ExternalOutput":
            out_names.append(name)
            shape = tuple(alloc.tensor_shape)
            dtype = mybir.dt.np(alloc.dtype)
            out_avals.append(jax.core.ShapedArray(shape, dtype))
            zero_outs.append(np.zeros(shape, dtype))
    n_params = len(in_names)
    n_outs = len(out_avals)
    all_in_names = list(in_names) + list(out_names)
    if partition_name is not None:
        all_in_names.append(partition_name)

    def _body(*args):
        operands = list(args)
        if partition_name is not None:
            operands.append(bass2jax.partition_id_tensor())
        outs = bass2jax._bass_exec_p.bind(
            *operands,
            out_avals=tuple(out_avals),
            in_names=tuple(all_in_names),
            out_names=tuple(out_names),
            lowering_input_output_aliases=(),
            sim_require_finite=True,
            sim_require_nnan=True,
            nc=nc,
        )
        return tuple(outs)

    devices = jax.devices()[:NCORES]
    mesh = Mesh(np.asarray(devices), ("core",))
    sharded = jax.jit(
        shard_map(
            _body,
            mesh=mesh,
            in_specs=(PartitionSpec("core"),) * (n_params + n_outs),
            out_specs=(PartitionSpec("core"),) * n_outs,
            check_rep=False,
        ),
        keep_unused=True,
    )

    def run(in_maps):
        concat_in = [
            np.concatenate([np.asarray(m[name]) for m in in_maps], axis=0)
            for name in in_names
        ]
        concat_zeros = [
            np.zeros((NCORES * z.shape[0], *z.shape[1:]), z.dtype) for z in zero_outs
        ]
        out_arrs = sharded(*concat_in, *concat_zeros)
        return [
            {
                name: np.asarray(out_arrs[i]).reshape(NCORES, *out_avals[i].shape)[c]
                for i, name in enumerate(out_names)
            }
            for c in range(NCORES)
        ]

    run.sharded = sharded
    run.in_names = in_names
    run.out_names = out_names
    run.out_avals = out_avals
    run.zero_outs = zero_outs
    return run


def prepare_in_maps(inputs):
    obs = np.asarray(inputs["obs"], dtype=np.float32).reshape(B, N, FIN)
    act = np.asarray(inputs["action"], dtype=np.float32).reshape(B, N, 2)
    x = np.concatenate([obs[..., 2:], act], axis=-1)  # [B, N, 10]
    xT = x.transpose(2, 0, 1).reshape(FIN, B * N)  # [10, B*N]

    NQ = NNODES // 4
    packs = np.zeros((NCORES, 128, PACK_COLS), np.float32)
    for c in range(NCORES):
        xc = xT[:, c * NNODES : (c + 1) * NNODES].reshape(FIN, 4, NQ)
        for q in range(4):
            packs[c, 32 * q : 32 * q + FIN, 0:NQ] = xc[:, q, :]
    # weights / biases identical on every core
    for idx, st in enumerate(("q1", "q2")):
        for li in range(3):
            fi = FIN if li == 0 else H
            wcol = NQ + (idx * 3 + li) * H
            packs[:, 0:fi, wcol : wcol + H] = np.asarray(inputs[f"{st}_w{li}"], np.float32)
            bcol = NQ + 6 * H + (idx * 3 + li)
            packs[:, :, bcol] = np.asarray(inputs[f"{st}_b{li}"], np.float32)
    mwcol = NQ + 6 * H + 6
    packs[:, :, mwcol] = np.asarray(inputs["mlp_w"], np.float32).reshape(H)
    packs[:, :, mwcol + 1] = np.float32(np.asarray(inputs["mlp_b"], np.float32).reshape(()))

    if MODE == "bf16":
        import ml_dtypes

        packs = packs.astype(ml_dtypes.bfloat16)

    return [{"inp": packs[c]} for c in range(NCORES)]


def kernel(**inputs):
    global LAST_RESULTS

    in_maps = prepare_in_maps(inputs)
    if MODE not in _CACHE:
        nc = _build(MODE)
        _CACHE[MODE] = (nc, _make_runner(nc))
    nc, runner = _CACHE[MODE]

    results = runner(in_maps)
    LAST_RESULTS = results

    def unpack(r, row0):
        # row j holds chunks c with c%4==j at free offset (c//4)*512
        o = np.asarray(r["q_out"])[row0 : row0 + 4].reshape(4, 8, 512)
        return o.transpose(1, 0, 2).reshape(S, N)

    q1 = np.concatenate([unpack(r, 0) for r in results], axis=0)
    q2 = np.concatenate([unpack(r, 4) for r in results], axis=0)
    return q1.astype(np.float32), q2.astype(np.float32)


if __name__ == "__main__":
    import reference as ref

    inputs = {k: np.asarray(v) for k, v in ref.setup_inputs().items()}
    q1, q2 = kernel(**inputs)
    print(q1[0, :8])


# revision 23
# speedup vs baseline: 4.5541x; 1.1027x over previous
"""Trainium2 Bass kernel for nn_GCNNDoubleQCritic (gnn_message_passing).

Key insight: the reference's knn is degenerate — `jnp.eye(N)*inf` makes the
whole distance matrix NaN (0*inf=NaN), and jax top_k sorts NaNs first (ties by
lower index). So node i's "neighbors" are simply the first 5 indices != i, the
in-degree is exactly 5 everywhere (deg=6 with self loop), and the GCN layer
collapses to, per sample:

    z   = h @ W
    c5  = z[0]+z[1]+z[2]+z[3]+z[4]
    out_i = relu((z_i + c5)/6 + b)   for i >= 6
    out_i = out_5                    for i <= 5     (since (z_5+c5) = sum_{j<=5} z_j)

The 1/6 is folded into the next layer's weights (relu is positively
homogeneous), so the on-chip layer is: G = relu(z + c5 + 6b), with W_{l>0} and
mlp_w pre-scaled by 1/6 on-chip.

Sharding: data-parallel over batch — 8 cores x 16 samples. Activations are
kept feature-major ([H, nodes]) the whole way so every matmul is a natural
lhsT=W[fi,fo], rhs=h_T[fi, n] contraction with zero transposes; the only
layout change (x -> x_T) is done host-side as part of input marshalling.
"""

import sys

sys.path.insert(0, "/opt/trn_rl_repo")

import numpy as np

B, N = 128, 1024
NCORES = 8
# flat packed input: 4 x-quarters of [10, 4096], then a [128, 782]-ish weight
# block flattened: 6 weights (128 cols each), 6 bias cols, mlp_w, mlp_b
XQ_ELEMS = 10 * 4096
WB_COLS = 6 * 128 + 8
PACK_ELEMS = 4 * XQ_ELEMS + 128 * WB_COLS
S = B // NCORES  # samples per core
NNODES = S * N  # nodes per core
H = 128
FIN = 10

# 'fp32' (exact, 4 cyc/row matmuls), 'f32r' (exact, 1 cyc/row), 'bf16'
MODE = "f32r"

_CACHE = {}
LAST_RESULTS = None


def _build(mode, repeat=1):
    import concourse.bacc as bacc
    import concourse.mybir as mybir
    from concourse.tile import TileContext

    fp = mybir.dt.float32
    bf = mybir.dt.bfloat16
    if mode == "bf16":
        act_dt = bf  # dtype h/x/weights are stored in (matmul operand dtype)
    elif mode == "f32r":
        act_dt = mybir.dt.float32r
    else:
        act_dt = fp

    nc = bacc.Bacc(
        "TRN2",
        target_bir_lowering=False,
        debug=False,
        enable_asserts=False,
        num_devices=NCORES,
    )

    in_dt = bf if mode == "bf16" else fp
    inp_d = nc.dram_tensor("inp", [PACK_ELEMS], in_dt, kind="ExternalInput")
    q_d = nc.dram_tensor("q_out", [8, 4096], fp, kind="ExternalOutput")

    with TileContext(nc) as tc:
        with (
            tc.tile_pool(name="const", bufs=1) as const,
            tc.tile_pool(name="hbuf", bufs=1) as hbuf,
            tc.tile_pool(name="small", bufs=8) as small,
            tc.tile_pool(name="zpsum", bufs=3, space="PSUM") as zpsum,
            tc.tile_pool(name="mpsum", bufs=2, space="PSUM") as mpsum,
        ):
            # ---- load the flat packed input ----
            NQ = NNODES // 4
            flat = inp_d.ap()
            xraw = const.tile([128, NQ], fp, tag="xraw")
            for q in range(4):
                nc.sync.dma_start(
                    xraw[32 * q : 32 * q + FIN, :],
                    flat[q * XQ_ELEMS : (q + 1) * XQ_ELEMS].rearrange(
                        "(a b) -> a b", a=FIN, b=NQ
                    ),
                )
            wb = const.tile([128, WB_COLS], fp, tag="wb")
            nc.sync.dma_start(
                wb[:],
                flat[4 * XQ_ELEMS :].rearrange("(a b) -> a b", a=128, b=WB_COLS),
            )

            if mode == "f32r":
                xTp = const.tile([128, NQ], act_dt, tag="xTp")
                nc.vector.tensor_copy(xTp[:], xraw[:])
            else:
                xTp = xraw

            w_sb = {}
            b6_sb = {}
            for idx, st in enumerate(("q1", "q2")):
                for li in range(3):
                    fi = FIN if li == 0 else H
                    wcol = (idx * 3 + li) * H
                    wsrc = wb[0:fi, wcol : wcol + H]
                    if li == 0:
                        # replicate w0 at partition offsets 0/32/64/96 for row tiling
                        wk = const.tile([128, H], act_dt, tag=f"w_{st}{li}")
                        for q in range(4):
                            nc.vector.tensor_copy(wk[32 * q : 32 * q + fi, :], wsrc)
                    else:
                        wk = const.tile([fi, H], act_dt, tag=f"w_{st}{li}")
                        nc.vector.tensor_scalar_mul(wk[:], wsrc, 1.0 / 6.0)
                    w_sb[st, li] = wk
                    bcol = 6 * H + (idx * 3 + li)
                    b6 = const.tile([H, 1], fp, tag=f"b6_{st}{li}")
                    nc.vector.tensor_scalar_mul(b6[:], wb[:, bcol : bcol + 1], 6.0)
                    b6_sb[st, li] = b6
            mwcol = 6 * H + 6
            mwscl = const.tile([H, 1], fp, tag="mwscl")
            nc.vector.tensor_scalar_mul(mwscl[:], wb[:, mwcol : mwcol + 1], 1.0 / 6.0)
            # mlp weight replicated across all 128 stationary columns: every
            # psum row of the head matmul equals q for that chunk
            mwrep = const.tile([H, H], act_dt, tag="mwrep")
            nc.vector.tensor_copy(mwrep[:], mwscl[:].to_broadcast([H, H]))
            # mlp_b replicated into every row host-side
            mb128 = wb[:, mwcol + 1 : mwcol + 2]

            hA = hbuf.tile([H, NNODES], act_dt, tag="hA")
            hB = hbuf.tile([H, NNODES], act_dt, tag="hB")
            qsb = const.tile([128, 4096], fp, tag="qsb", name="qsb")

            for rep in range(repeat):
              for st in ("q1", "q2"):
                for li in range(3):
                    h_in = (hA if li % 2 == 1 else hB) if li > 0 else None
                    h_out = hA if li % 2 == 0 else hB
                    wk = w_sb[st, li]
                    b6 = b6_sb[st, li]
                    for s in range(S):
                        zT = zpsum.tile([H, N], fp, tag="zT")
                        for half in range(2):
                            if li == 0:
                                q4 = s // 4
                                c0 = (s % 4) * N + half * 512
                                nc.tensor.matmul(
                                    zT[:, half * 512 : (half + 1) * 512],
                                    wk[32 * q4 : 32 * q4 + FIN, :],
                                    xTp[32 * q4 : 32 * q4 + FIN, c0 : c0 + 512],
                                    start=True,
                                    stop=True,
                                    tile_position=(32 * q4, 0),
                                )
                            else:
                                c0 = s * N + half * 512
                                nc.tensor.matmul(
                                    zT[:, half * 512 : (half + 1) * 512],
                                    wk[:],
                                    h_in[:, c0 : c0 + 512],
                                    start=True,
                                    stop=True,
                                )
                        bias_s = small.tile([H, 1], fp, tag="bias_s")
                        # bias_s = sum(z cols 0..4) + 6b
                        c5 = small.tile([H, 1], fp, tag="c5")
                        nc.vector.tensor_reduce(
                            c5[:], zT[:, 0:5], axis=mybir.AxisListType.X, op=mybir.AluOpType.add
                        )
                        nc.vector.tensor_add(bias_s[:], c5[:], b6[:])
                        seg = h_out[:, s * N : (s + 1) * N]
                        if s % 2 == 0:
                            nc.scalar.activation(
                                seg,
                                zT[:],
                                mybir.ActivationFunctionType.Relu,
                                bias=bias_s[:],
                                scale=1.0,
                            )
                        else:
                            nc.vector.tensor_scalar(
                                seg,
                                zT[:],
                                bias_s[:],
                                0.0,
                                op0=mybir.AluOpType.add,
                                op1=mybir.AluOpType.max,
                            )
                        # nodes 0..4 take node 5's value
                        nc.vector.tensor_copy(
                            h_out[:, s * N : s * N + 5],
                            h_out[:, s * N + 5 : s * N + 6].to_broadcast([H, 5]),
                        )
                    h_in = h_out

                # ---- output head: q = h3 @ (mlp_w/6) + mlp_b ----
                h3 = hA
                for c in range(NNODES // 512):  # 32 chunks
                    mp = mpsum.tile([H, 512], fp, tag="mp")
                    nc.tensor.matmul(
                        mp[:], mwrep[:], h3[:, c * 512 : (c + 1) * 512], start=True, stop=True
                    )
                    # every row of mp equals q[chunk c]; evac from a 32-aligned
                    # partition (PSUM engine access must be 32-aligned)
                    p = 32 * (c % 4)
                    g = c // 4
                    if c % 2 == 0:
                        nc.vector.tensor_scalar_add(
                            qsb[p : p + 1, g * 512 : (g + 1) * 512],
                            mp[p : p + 1, :],
                            mb128[p : p + 1, :],
                        )
                    else:
                        nc.scalar.activation(
                            qsb[p : p + 1, g * 512 : (g + 1) * 512],
                            mp[p : p + 1, :],
                            mybir.ActivationFunctionType.Identity,
                            bias=mb128[p : p + 1, :],
                            scale=1.0,
                        )
                row0 = 0 if st == "q1" else 4
                for j in range(4):
                    nc.sync.dma_start(
                        q_d.ap()[row0 + j : row0 + j + 1, :], qsb[32 * j : 32 * j + 1, :]
                    )

    nc.compile()
    return nc


def _make_runner(nc):
    """Like bass2jax.run_bass_via_pjrt, but returns a reusable jitted callable
    so repeat calls skip retracing/relowering."""
    import jax
    import numpy as np
    from jax.sharding import Mesh, PartitionSpec
    from jax.experimental.shard_map import shard_map

    import concourse.mybir as mybir
    from concourse import bass2jax

    bass2jax.install_neuronx_cc_hook()

    partition_name = nc.partition_id_tensor.name if nc.partition_id_tensor else None
    in_names, out_names, out_avals, zero_outs = [], [], [], []
    for alloc in nc.m.functions[0].allocations:
        if not isinstance(alloc, mybir.MemoryLocationSet):
            continue
        name = alloc.memorylocations[0].name
        if alloc.kind == "ExternalInput":
            if name != partition_name:
                in_names.append(name)
        elif alloc.kind == "ExternalOutput":
            out_names.append(name)
            shape = tuple(alloc.tensor_shape)
            dtype = mybir.dt.np(alloc.dtype)
            out_avals.append(jax.core.ShapedArray(shape, dtype))
            zero_outs.append(np.zeros(shape, dtype))
    n_params = len(in_names)
    n_outs = len(out_avals)
    all_in_names = list(in_names) + list(out_names)
    if partition_name is not None:
        all_in_names.append(partition_name)

    def _body(*args):
        operands = list(args)
        if partition_name is not None:
            operands.append(bass2jax.partition_id_tensor())
        outs = bass2jax._bass_exec_p.bind(
            *operands,
            out_avals=tuple(out_avals),
            in_names=tuple(all_in_names),
            out_names=tuple(out_names),
            lowering_input_output_aliases=(),
            sim_require_finite=True,
            sim_require_nnan=True,
            nc=nc,
        )
        return tuple(outs)

    devices = jax.devices()[:NCORES]
    mesh = Mesh(np.asarray(devices), ("core",))
    sharded = jax.jit(
        shard_map(
            _body,
            mesh=mesh,
            in_specs=(PartitionSpec("core"),) * (n_params + n_outs),
            out_specs=(PartitionSpec("core"),) * n_outs,
            check_rep=False,
        ),
        keep_unused=True,
    )

    def run(in_maps):
        concat_in = [
            np.concatenate([np.asarray(m[name]) for m in in_maps], axis=0)
            for name in in_names
        ]
        concat_zeros = [
            np.zeros((NCORES * z.shape[0], *z.shape[1:]), z.dtype) for z in zero_outs
        ]
        out_arrs = sharded(*concat_in, *concat_zeros)
        return [
            {
                name: np.asarray(out_arrs[i]).reshape(NCORES, *out_avals[i].shape)[c]
                for i, name in enumerate(out_names)
            }
            for c in range(NCORES)
        ]

    run.sharded = sharded
    run.in_names = in_names
    run.out_names = out_names
    run.out_avals = out_avals
    run.zero_outs = zero_outs
    return run


def prepare_in_maps(inputs):
    obs = np.asarray(inputs["obs"], dtype=np.float32).reshape(B, N, FIN)
    act = np.asarray(inputs["action"], dtype=np.float32).reshape(B, N, 2)
    x = np.concatenate([obs[..., 2:], act], axis=-1)  # [B, N, 10]
    xT = x.transpose(2, 0, 1).reshape(FIN, B * N)  # [10, B*N]

    NQ = NNODES // 4
    wb = np.zeros((128, WB_COLS), np.float32)
    for idx, st in enumerate(("q1", "q2")):
        for li in range(3):
            fi = FIN if li == 0 else H
            wcol = (idx * 3 + li) * H
            wb[0:fi, wcol : wcol + H] = np.asarray(inputs[f"{st}_w{li}"], np.float32)
            wb[:, 6 * H + (idx * 3 + li)] = np.asarray(inputs[f"{st}_b{li}"], np.float32)
    wb[:, 6 * H + 6] = np.asarray(inputs["mlp_w"], np.float32).reshape(H)
    wb[:, 6 * H + 7] = np.float32(np.asarray(inputs["mlp_b"], np.float32).reshape(()))
    wb_flat = wb.reshape(-1)

    packs = np.empty((NCORES, PACK_ELEMS), np.float32)
    for c in range(NCORES):
        xc = xT[:, c * NNODES : (c + 1) * NNODES].reshape(FIN, 4, NQ)
        packs[c, : 4 * XQ_ELEMS] = xc.transpose(1, 0, 2).reshape(-1)
        packs[c, 4 * XQ_ELEMS :] = wb_flat

    if MODE == "bf16":
        import ml_dtypes

        packs = packs.astype(ml_dtypes.bfloat16)

    return [{"inp": packs[c]} for c in range(NCORES)]


def kernel(**inputs):
    global LAST_RESULTS

    in_maps = prepare_in_maps(inputs)
    if MODE not in _CACHE:
        nc = _build(MODE)
        _CACHE[MODE] = (nc, _make_runner(nc))
    nc, runner = _CACHE[MODE]

    results = runner(in_maps)
    LAST_RESULTS = results

    def unpack(r, row0):
        # row j holds chunks c with c%4==j at free offset (c//4)*512
        o = np.asarray(r["q_out"])[row0 : row0 + 4].reshape(4, 8, 512)
        return o.transpose(1, 0, 2).reshape(S, N)

    q1 = np.concatenate([unpack(r, 0) for r in results], axis=0)
    q2 = np.concatenate([unpack(r, 4) for r in results], axis=0)
    return q1.astype(np.float32), q2.astype(np.float32)


if __name__ == "__main__":
    import reference as ref

    inputs = {k: np.asarray(v) for k, v in ref.setup_inputs().items()}
    q1, q2 = kernel(**inputs)
    print(q1[0, :8])
